# revision 6
# baseline (speedup 1.0000x reference)
"""Trainium2 Bass kernel for nn_BatchContrastLoss (InfoNCE-style contrastive loss).

Reference computation:
    sim[i,j]  = cos(que_i, ans_j)            (eps-guarded norms)
    logits    = sim / 0.07
    loss      = -mean_i(log_softmax(logits, axis=1)[i,i])

Key restructuring vs the straightforward port: cosine normalization is LINEAR
in each operand, so (q_i/(gama*|q_i|)) . (a_j/|a_j|) == logits_ij exactly.
The row/column norms are folded into the host-side fp8 quantization pass that
already has to touch every element. The device then runs only the two
irreducible parts -- the [B/4, B/2] fp8 GEMM slab and the row-wise
exp-accumulate -- and everything else (log, diagonal dot, mean) stays on the
host where it is O(B*D) noise.

Sharding: 2D (4 que-shards x 2 ans-halves) over 8 cores. Each core reads a
1MB que slab + 2MB ans half (vs 4.5MB for 1D row sharding), computes its
[1024, 2048] logits block, and emits per-row-tile exp-sums. Host pairs the
two ans-halves per row (a trivial add), takes log, subtracts the host-computed
diagonal logits, and means. No cross-core collective (rank-skew stalls cost
more than the 4KB/core of extra host traffic).

Device-side structure (PE-roofline bound: ~27us of DoubleRow fp8 matmul):
  - operands arrive pre-paired d-major [128, 2, *] for DoubleRow e4m3 matmuls
    (2 weights/cell, K=256 per instruction), in exactly the SBUF layout so
    every DMA descriptor is a contiguous 1-2KB per-partition run. Transfers
    are partition-split across rings (a single ring moves only ~36GB/s) and
    doorbells alternate between the two HWDGE engines (SP/Act) because each
    ring-ring costs ~0.6us of sequencer time.
  - each [128, 2, 128] weight tile is loaded ONCE (explicit ldweights +
    non-self-loading matmuls) and reused across the 4 column chunks; a
    self-loading matmul stream spends ~40% of the PE on redundant LDWEIGHTS.
  - the first k-sweep interleaves row-tiles m0+m1 (8 PSUM banks) so the PE
    consumption rate (~8 matmuls per 512KB ans block) matches the DMA arrival
    rate; later row-tiles run from resident SBUF at full PE rate.
  - a short warm-up matmul chain on memset tiles spins the PE p-state up
    (0.65 -> 2.4GHz takes ~3us of continuous busy) and a dummy activation
    pre-loads the Exp table (1.3us) while the first DMAs are in flight.
  - drains are ScalarE Exp over 2-bank [128, 1024] PSUM spans with the fused
    row-sum accumulator; the log/diag/mean run on the host.
"""

import numpy as np

import concourse.bass as bass
import concourse.mybir as mybir
import concourse.tile as tile
from concourse import bacc
from concourse.bass_utils import run_bass_kernel_spmd

# Problem constants (self-contained; the harness provides only the inputs).
B = 4096  # rows of que_batch / ans_batch
D = 1024  # feature dim
NCORES = 8
RSH = 4  # que row shards
CSH = 2  # ans column shards
MB = B // RSH  # local que rows per core = 1024
NB = B // CSH  # local ans cols per core = 2048
P = 128  # SBUF partitions
KT2 = D // (2 * P)  # 4 DoubleRow k-pair tiles (K=256 each)
NW = 512  # column chunk width (one fp32 PSUM bank)
NCH = NB // NW  # 4 column chunks
MT = MB // P  # 8 row tiles of 128
GAMA = 0.07
EPS = 1e-8
NWARM = 24  # PE p-state warm-up matmuls

F32 = mybir.dt.float32
FP8 = mybir.dt.float8e4  # e4m3: matmul operands; DoubleRow packs 2 weights/cell
DR = mybir.MatmulPerfMode.DoubleRow
AF = mybir.ActivationFunctionType


def _build_program():
    nc = bacc.Bacc(
        "TRN2", target_bir_lowering=False, debug=False, num_devices=NCORES
    )

    # Host-prepped layouts (fp8, DoubleRow-paired, d-major):
    #   qdr[p, t, i, m]     = qhat[m_local, d=(2t+i)*128+p]
    #   adr[p, t, n, i, j2] = ahat[n*512+j2 local, d=(2t+i)*128+p]
    qdr = nc.dram_tensor("qdr", [P, KT2, 2, MB], FP8, kind="ExternalInput").ap()
    adr = nc.dram_tensor("adr", [P, KT2, NCH, 2, NW], FP8, kind="ExternalInput").ap()
    s_out = nc.dram_tensor("s_out", [P, MT * 2], F32, kind="ExternalOutput").ap()

    with tile.TileContext(nc) as tc:
        with (
            tc.tile_pool(name="persist", bufs=1) as persist,
            tc.tile_pool(name="work", bufs=2) as work,
            tc.tile_pool(name="psp", bufs=2, space="PSUM") as psp,
        ):
            _body(nc, persist, work, psp, qdr, adr, s_out)

    nc.compile()
    return nc


def _mm(nc, out, lhsT, rhs, start, stop):
    inst = nc.tensor.matmul(
        out, lhsT=lhsT, rhs=rhs, start=start, stop=stop, perf_mode=DR
    )
    inst.ldweights = False  # weights come from the preceding explicit ldweights
    return inst


def _body(nc, persist, work, psp, qdr, adr, s_out):
    # ---- PE p-state warm-up + Exp act-table preload, all on memset tiles,
    # while the first DMAs are still in flight.
    wl = persist.tile([P, 2, P], FP8, tag="wl")
    nc.vector.memset(wl, 0.25)
    wdum = persist.tile([P, 1], F32, tag="wdum")
    nc.vector.memset(wdum, 0.0)
    sdum = work.tile([P, 1], F32, tag="sdum", bufs=1)
    nc.scalar.activation(sdum, wdum, AF.Exp)  # pulls the Exp table in early
    # warm-up psum shares the rotating "ps" tag; the warm-up chain is first
    # in PE queue order, so the later tile reusing this slot never stalls.
    wps = psp.tile([P, NCH, NW], F32, tag="ps", bufs=2, name="wps")
    for w in range(NWARM):
        nc.tensor.matmul(
            wps[:, 0, 0:P], lhsT=wl, rhs=wl, start=True, stop=True, perf_mode=DR
        )

    # ---- DMA front. Partition-halved transfers (two rings each) so no
    # single ~36GB/s ring gates the critical path; issue order follows the
    # m0/m1 consumption order; doorbells alternate sync/scalar (each ring
    # costs ~0.6us of sequencer time).
    db = [nc.sync, nc.scalar]
    ndb = 0

    def dma(out_ap, in_ap):
        nonlocal ndb
        db[ndb % 2].dma_start(out=out_ap, in_=in_ap)
        ndb += 1

    qts = []
    ats = {}
    for t in range(KT2):
        qt = persist.tile([P, 2, MB], FP8, tag=f"q{t}", name=f"q{t}")
        qts.append(qt)
        for h in range(2):
            pr = slice(h * 64, (h + 1) * 64)
            dma(qt[pr], qdr[pr, t])
        for n in range(NCH):
            a = persist.tile([P, 2, NW], FP8, tag=f"a{t}_{n}", name=f"a{t}_{n}")
            ats[(t, n)] = a
            for h in range(2):
                pr = slice(h * 64, (h + 1) * 64)
                dma(a[pr], adr[pr, t, n])

    s_sb = persist.tile([P, MT * 2], F32, tag="s_sb")

    def mm_group(m, t, ps):
        w = qts[t][:, :, m * P : (m + 1) * P]
        nc.tensor.ldweights(w, perf_mode=DR)
        for n in range(NCH):
            _mm(nc, ps[:, n], w, ats[(t, n)], start=(t == 0), stop=(t == KT2 - 1))

    def drain(m, ps):
        # two Exp instructions per row tile, each spanning 2 PSUM banks,
        # with fused row-sum accumulation; host adds the column pairs.
        for h in range(2):
            scr = work.tile(
                [P, 2, NW], F32, tag="scr", bufs=4, name=f"scr_{m}_{h}"
            )
            nc.scalar.activation(
                scr,
                ps[:, 2 * h : 2 * h + 2],
                AF.Exp,
                accum_out=s_sb[:, 2 * m + h : 2 * m + h + 1],
            )

    # ---- first k-sweep: m0+m1 interleaved so the PE tracks the DMA stream.
    ps0 = psp.tile([P, NCH, NW], F32, tag="ps", bufs=2, name="ps_0")
    ps1 = psp.tile([P, NCH, NW], F32, tag="ps", bufs=2, name="ps_1")
    for t in range(KT2):
        mm_group(0, t, ps0)
        mm_group(1, t, ps1)
    drain(0, ps0)
    drain(1, ps1)

    # ---- remaining row tiles from resident SBUF.
    for m in range(2, MT):
        ps = psp.tile([P, NCH, NW], F32, tag="ps", bufs=2, name=f"ps_{m}")
        for t in range(KT2):
            mm_group(m, t, ps)
        drain(m, ps)

    nc.sync.dma_start(out=s_out, in_=s_sb)


_CACHE = {}


def _get_program():
    if "nc" not in _CACHE:
        _CACHE["nc"] = _build_program()
    return _CACHE["nc"]


def _prep(que, ans):
    """Normalize (norm folding), quantize to fp8, lay out for DoubleRow DMA.

    Returns (in_maps, diag) where diag[i] = qhat_i . ahat_i computed from the
    exact fp8 values the device multiplies (f32 accumulation, same as PSUM).
    """
    fp8 = mybir.dt.np(FP8)
    que = np.asarray(que, dtype=np.float32)
    ans = np.asarray(ans, dtype=np.float32)
    qn = np.maximum(np.sqrt(np.einsum("id,id->i", que, que)), EPS)
    an = np.maximum(np.sqrt(np.einsum("id,id->i", ans, ans)), EPS)
    qhat = (que / (np.float32(GAMA) * qn)[:, None]).astype(fp8)
    ahat = (ans / an[:, None]).astype(fp8)

    qf = qhat.astype(np.float32)
    af = ahat.astype(np.float32)
    diag = np.einsum("id,id->i", qf, af)  # logits diagonal, bit-compatible

    in_maps = []
    for cid in range(NCORES):
        r, c = divmod(cid, CSH)
        qslab = qhat[r * MB : (r + 1) * MB]  # [MB, D]
        aslab = ahat[c * NB : (c + 1) * NB]  # [NB, D]
        # [D, MB] -> [KT2, 2, P, MB] -> [P, KT2, 2, MB]
        qdr = np.ascontiguousarray(
            qslab.T.reshape(KT2, 2, P, MB).transpose(2, 0, 1, 3)
        )
        # [D, NB] -> [KT2, 2, P, NCH, NW] -> [P, KT2, NCH, 2, NW]
        adr = np.ascontiguousarray(
            aslab.T.reshape(KT2, 2, P, NCH, NW).transpose(2, 0, 3, 1, 4)
        )
        in_maps.append({"qdr": qdr, "adr": adr})
    return in_maps, diag


def _finish(results, diag):
    # s_out[p, 2m+h] = sum_j exp(logits) over half h of this core's ans
    # half, local row m*128+p.
    s = np.zeros(B, dtype=np.float64)
    for cid, res in enumerate(results):
        r, _ = divmod(cid, CSH)
        so = np.asarray(res["s_out"], dtype=np.float64)  # [P, MT*2]
        for m in range(MT):
            base = r * MB + m * P
            s[base : base + P] += so[:, 2 * m] + so[:, 2 * m + 1]
    loss = np.float32(np.mean(np.log(s) - diag))
    return np.array([loss], dtype=np.float32)


def kernel(que_batch, ans_batch):
    nc = _get_program()
    in_maps, diag = _prep(que_batch, ans_batch)
    res = run_bass_kernel_spmd(nc, in_maps, list(range(NCORES)))
    return _finish(res.results, diag)


if __name__ == "__main__":
    rng = np.random.default_rng(0)
    q = rng.standard_normal((B, D), dtype=np.float32)
    a = rng.standard_normal((B, D), dtype=np.float32)
    print(kernel(q, a))
